# revision 18
# baseline (speedup 1.0000x reference)
"""Trainium2 Bass kernel for nn_DeltaModel (delta-rule recurrence LM head).

Math (validated against the reference in numpy):
  * Every per-token quantity depends only on the token id (VOCAB=64):
      hs_vocab = LN(embed + FFN(embed))              [64, 64]
      kn_vocab = l2norm(hs_vocab @ wk), v_vocab = hs_vocab @ wv,
      q_vocab  = hs_vocab @ wq
  * Only the final position is queried, so the O(T*H^2) matrix recurrence
    collapses to an O(T*H) backward vector scan:
      p_T = q;  c_t = k_t . p_t;  p_{t-1} = p_t - c_t k_t   (t = T..1)
      r = sum_t c_t v_t
      out = r @ (wrp @ wout) + (brp @ wout + bout)

Mapping: pure data parallel over batch (B=256 -> 8 cores x 32).  Per core:
  - vocab tables computed on-chip (tiny matmuls), written to internal DRAM
  - k table stored fp16 (rows padded to 256B); dma_gather streams
    per-(b,t) kn rows into [b x H] fp16 tiles
  - the scan runs as 2 fused DVE ops per step (scalar_tensor_tensor dot
    with accum + scalar_tensor_tensor axpy) over [32 x 64] fp16 tiles
  - r = sum_t c_t v_t accumulated on the PE: v rows gathered time-major
    ([128(t), 32(b), 64]), c chunks transposed via PE, then per-(b,chunk)
    [128x64]^T @ [128x1] matmuls accumulate into one PSUM tile
  - head: two small matmuls (folded wrp@wout), bias add, dma out
"""

from contextlib import ExitStack

import numpy as np

import concourse.bass as bass
import concourse.bacc as bacc
import concourse.mybir as mybir
import concourse.tile as tile
from concourse import library_config
from concourse.bass_utils import run_bass_kernel_spmd

F32 = mybir.dt.float32
F16 = mybir.dt.float16
I32 = mybir.dt.int32
I16 = mybir.dt.int16
OP = mybir.AluOpType
ACT = mybir.ActivationFunctionType

H = 64
V = 64
L = 2048
T = L - 1          # 2047 recurrence steps
B = 256
NCORES = 8
BL = B // NCORES   # 32 sequences per core
GBLK = 64          # timesteps per kn gather call
NBLK = L // GBLK   # 32 kn gather blocks
C = 128            # r-accumulation chunk (cb transpose granularity)
NC = L // C        # 16 chunks
LN_EPS = 1e-5
NORM_EPS = 1e-12

WNAMES = ["embed", "w1", "b1", "w2", "b2", "ln_g", "ln_b", "wk", "wv", "wq",
          "wrp", "brp", "wout", "bout"]

_CACHE = {}
import os
_LVL = int(os.environ.get("K_LVL", "3"))


def _body(nc, tc, d):
    ctx = ExitStack()
    sg = ctx.enter_context(tc.tile_pool(name="sg", bufs=1))
    ps = ctx.enter_context(tc.tile_pool(name="ps", bufs=2, space="PSUM"))
    rp = ctx.enter_context(tc.tile_pool(name="rp", bufs=1, space="PSUM"))
    kgp = ctx.enter_context(tc.tile_pool(name="kgp", bufs=3))
    vgp = ctx.enter_context(tc.tile_pool(name="vgp", bufs=2))

    # ---- load weights to SBUF ----
    w = {}
    shapes = {"embed": [V, H], "w1": [H, 2 * H], "w2": [2 * H, H],
              "wk": [H, H], "wv": [H, H], "wq": [H, H], "wrp": [H, H],
              "wout": [H, V]}
    for n, shp in shapes.items():
        w[n] = sg.tile(shp, F32, tag=f"w_{n}", name=f"w_{n}")
        nc.sync.dma_start(out=w[n], in_=d[n])
    cols = {"b1": 2 * H, "b2": H, "brp": H, "bout": V}
    for n, p in cols.items():
        w[n] = sg.tile([p, 1], F32, tag=f"w_{n}", name=f"w_{n}")
        col = bass.AP(tensor=d[n].tensor, offset=d[n].offset,
                      ap=[list(d[n].ap[0]), [1, 1]])
        nc.sync.dma_start(out=w[n], in_=col)
    # ln_g/ln_b broadcast along partitions: [64v x 64h]
    for n in ("ln_g", "ln_b"):
        w[n] = sg.tile([V, H], F32, tag=f"w_{n}", name=f"w_{n}")
        bcast = bass.AP(tensor=d[n].tensor, offset=d[n].offset,
                        ap=[[0, V]] + list(d[n].ap))
        nc.gpsimd.dma_start(out=w[n], in_=bcast)

    # ---- identity for PE transposes ----
    iota_row = sg.tile([128, 128], mybir.dt.int32)
    nc.gpsimd.iota(iota_row, pattern=[[1, 128]], base=0, channel_multiplier=0)
    iota_col = sg.tile([128, 1], mybir.dt.int32)
    nc.gpsimd.iota(iota_col, pattern=[[0, 1]], base=0, channel_multiplier=1)
    iota_row_f = sg.tile([128, 128], F32)
    nc.vector.tensor_copy(iota_row_f, iota_row)
    iota_col_f = sg.tile([128, 1], F32)
    nc.vector.tensor_copy(iota_col_f, iota_col)
    ident = sg.tile([128, 128], F32)
    nc.vector.tensor_scalar(out=ident, in0=iota_row_f, scalar1=iota_col_f,
                            scalar2=None, op0=OP.is_equal)

    def transpose(src, p, f):
        """src [p x f] sbuf -> returns [f x p] sbuf tile."""
        pt = ps.tile([f, p], F32, tag="ps", name="tr_ps")
        nc.tensor.transpose(pt, src, ident[0:p, 0:p])
        out = sg.tile([f, p], F32, tag=f"tr_{src.tensor.name}",
                      name=f"tr_{src.tensor.name}")
        nc.vector.tensor_copy(out, pt)
        return out

    # ---- vocab pipeline ----
    eT = transpose(w["embed"], V, H)                     # [h x v]
    u1p = ps.tile([2 * H, V], F32, tag="ps")
    nc.tensor.matmul(u1p, w["w1"], eT, start=True, stop=True)
    u1T = sg.tile([2 * H, V], F32)
    nc.scalar.activation(u1T, u1p, ACT.Relu, bias=w["b1"], scale=1.0)
    fp = ps.tile([H, V], F32, tag="ps")
    nc.tensor.matmul(fp, w["w2"], u1T, start=True, stop=True)
    xT = sg.tile([H, V], F32)
    nc.vector.scalar_tensor_tensor(out=xT, in0=fp, scalar=w["b2"], in1=eT,
                                   op0=OP.add, op1=OP.add)
    x = transpose(xT, H, V)                              # [v x h]
    # layernorm over h (free axis)
    stats = sg.tile([V, 6], F32)
    nc.vector.bn_stats(out=stats, in_=x)
    mv = sg.tile([V, 2], F32)
    nc.vector.bn_aggr(out=mv, in_=stats)
    rstd = sg.tile([V, 1], F32)
    nc.vector.tensor_scalar_add(rstd, mv[:, 1:2], LN_EPS)
    nc.scalar.sqrt(rstd, rstd)
    nc.vector.reciprocal(rstd, rstd)
    hs = sg.tile([V, H], F32)
    nc.vector.tensor_scalar(out=hs, in0=x, scalar1=mv[:, 0:1],
                            scalar2=rstd, op0=OP.subtract, op1=OP.mult)
    nc.vector.tensor_mul(hs, hs, w["ln_g"])
    nc.vector.tensor_add(hs, hs, w["ln_b"])
    hsT = transpose(hs, V, H)                            # [h x v]

    def vocab_mm(wname):
        pt = ps.tile([V, H], F32, tag="ps", name="vmm_ps")
        nc.tensor.matmul(pt, hsT, w[wname], start=True, stop=True)
        out = sg.tile([V, H], F32, tag=f"vmm_{wname}", name=f"vmm_{wname}")
        nc.vector.tensor_copy(out, pt)
        return out

    kv = vocab_mm("wk")                                  # [v x h] (pre-norm k)
    vv = vocab_mm("wv")
    qv = vocab_mm("wq")
    # l2-normalize rows of kv
    ksq = sg.tile([V, H], F32)
    ss = sg.tile([V, 1], F32)
    nc.scalar.activation(ksq, kv, ACT.Square, accum_out=ss)
    nrm = sg.tile([V, 1], F32)
    nc.scalar.sqrt(nrm, ss)
    nc.vector.tensor_scalar_max(nrm, nrm, NORM_EPS)
    nc.vector.reciprocal(nrm, nrm)
    kn16 = sg.tile([V, 2 * H], F16)      # fp16 k rows padded to 256B
    nc.vector.memset(kn16, 0.0)
    nc.vector.tensor_scalar_mul(kn16[:, 0:H], kv, nrm)
    # output head fold: WO = wrp @ wout; BO = (brp @ wout + bout)^T
    wrpT = transpose(w["wrp"], H, H)
    WOp = ps.tile([H, V], F32, tag="ps")
    nc.tensor.matmul(WOp, wrpT, w["wout"], start=True, stop=True)
    WO = sg.tile([H, V], F32)
    nc.vector.tensor_copy(WO, WOp)
    BOp = ps.tile([V, 1], F32, tag="ps")
    nc.tensor.matmul(BOp, w["wout"], w["brp"], start=True, stop=True)
    BO = sg.tile([V, 1], F32)
    nc.vector.tensor_add(BO, BOp, w["bout"])

    # ---- vocab tables to DRAM for dma_gather ----
    ktab = nc.dram_tensor("ktab", [V, 2 * H], F16, kind="Internal").ap()
    vtab = nc.dram_tensor("vtab", [V, H], F32, kind="Internal").ap()
    qtab = nc.dram_tensor("qtab", [V, H], F32, kind="Internal").ap()
    nc.sync.dma_start(out=ktab, in_=kn16)
    nc.sync.dma_start(out=vtab, in_=vv)
    nc.sync.dma_start(out=qtab, in_=qv)

    # ---- seq to SBUF ----
    seq_sb = sg.tile([BL, L], I32)
    nc.sync.dma_start(out=seq_sb, in_=d["seq"])
    seqf = sg.tile([BL, L], F32)
    nc.vector.tensor_copy(seqf, seq_sb)

    # seq rows staged on partitions 0-15 (DVE reads must start at partition
    # 0/32/64/96, so partition-16 slices go through DMA-staged tiles)
    seq16 = []
    for m in range(2):
        s16 = sg.tile([16, L], I32, tag=f"seq16_{m}", name=f"seq16_{m}")
        nc.gpsimd.dma_start(out=s16, in_=d["seq"][16 * m:16 * (m + 1), :])
        seq16.append(s16)

    # ---- kn gather indices (b-major, baseline layout) ----
    # gather j = t*128 + b: out partition b, free slot t.  idx tile
    # [16 part, col]: col = t*8 + (b//16), part = b%16, value = seq[b, t];
    # cols t*8 + {2..7} stay 0 (partitions 32-127 gather row 0, unused).
    idx_k = sg.tile([128, L * 8], I16)
    nc.vector.memset(idx_k, 0)
    for m in range(2):
        src = seq16[m]                                        # [16, 2048]
        dst = idx_k[0:16, m::8]                               # [16, 2048]
        nc.vector.tensor_copy(dst, src)
    # replicate group 0 -> groups 1..7, column-blocked in reverse scan order
    # (the scan consumes high blocks first) and spread across engine queues
    # so the first kn gather isn't gated on one serialized queue.
    dma_engines = [nc.sync, nc.gpsimd, nc.scalar]
    for i, kb in enumerate(reversed(range(NBLK))):
        lo, hi = GBLK * 8 * kb, GBLK * 8 * (kb + 1)
        for g in range(1, 8):
            eng = dma_engines[(i * 7 + g - 1) % len(dma_engines)]
            eng.dma_start(out=idx_k[16 * g:16 * (g + 1), lo:hi],
                          in_=idx_k[0:16, lo:hi])

    # ---- q gather index (final token, t = L-1) ----
    idx_q = sg.tile([128, 8], I16)
    nc.vector.memset(idx_q, 0)
    for m in range(2):
        nc.vector.tensor_copy(idx_q[0:16, m:m + 1],
                              seq16[m][:, L - 1:L])
    for g in range(1, 8):
        nc.sync.dma_start(out=idx_q[16 * g:16 * (g + 1), :],
                          in_=idx_q[0:16, :])

    # ---- v gather indices (time-major): j = b*128 + t_in_chunk ----
    # out partition = t_in_chunk, free = b.  idx tile col = b*8 + (t//16),
    # part = t%16, value = seq[b, chunk*128 + t] -> needs seq transposed.
    idx_v = sg.tile([128, NC * BL * 8], I16)

    def _build_idx_v(blk):
        stp = ps.tile([C, BL], F32, tag="ps", name="seqT_ps")
        nc.tensor.transpose(stp, seqf[:, blk * C:(blk + 1) * C],
                            ident[0:BL, 0:BL])
        seqT_i = sg.tile([C, BL], I16, tag="seqT_i", name="seqT_i")
        nc.vector.tensor_copy(seqT_i, stp)
        base = blk * BL * 8
        for m in range(8):
            dst = idx_v[0:16, base + m:base + BL * 8:8]       # [16, 32]
            eng = dma_engines[(blk * 8 + m) % len(dma_engines)]
            eng.dma_start(out=dst, in_=seqT_i[16 * m:16 * (m + 1), :])
    for blk in range(NC):
        _build_idx_v(blk)
    for g in range(1, 8):
        eng = dma_engines[(g - 1) % len(dma_engines)]
        eng.dma_start(out=idx_v[16 * g:16 * (g + 1), :],
                      in_=idx_v[0:16, :])

    # ---- q gather -> p init (fp16) ----
    qg = kgp.tile([128, 1, H], F32, tag="qg", name="qg")
    nc.gpsimd.dma_gather(qg, qtab, idx_q, 128, 128, H)
    p16 = sg.tile([BL, H], F16)
    nc.vector.tensor_copy(p16, qg[0:BL, 0, :])

    # ---- backward scan (fp16, 2 DVE ops/step) ----
    cb = sg.tile([BL, L], F32)        # stores -c_t; col T unused (zero)
    if _LVL == 0:
        nc.vector.memset(cb, 0.0)
    else:
        nc.vector.memset(cb[:, T:T + 1], 0.0)
    scr16 = sg.tile([BL, H], F16)
    for kb in reversed(range(NBLK)):
        lo = GBLK * kb
        kg = kgp.tile([128, GBLK, 2 * H], F16, tag="kg", name="kg")
        nc.gpsimd.dma_gather(kg, ktab, idx_k[:, lo * 8:(lo + GBLK) * 8],
                             GBLK * 128, GBLK * 128, 2 * H,
                             single_packet=False)
        t_hi = min(GBLK * (kb + 1), T)
        if _LVL == 0:
            continue
        for t in reversed(range(lo, t_hi)):
            kt = kg[0:BL, t - lo, 0:H]
            nc.vector.scalar_tensor_tensor(
                out=scr16, in0=kt, scalar=-1.0, in1=p16,
                op0=OP.mult, op1=OP.mult, accum_out=cb[:, t:t + 1])
            nc.vector.scalar_tensor_tensor(
                out=p16, in0=kt, scalar=cb[:, t:t + 1], in1=p16,
                op0=OP.mult, op1=OP.add)

    # ---- r = sum_t c_t v_t via per-chunk PE matmuls ----
    racc = sg.tile([H, BL], F32)      # accumulates -r
    nc.vector.memset(racc, 0.0)
    for blk in (reversed(range(NC)) if _LVL >= 2 else []):
        vg = vgp.tile([128, BL, H], F32, tag="vg", name="vg")
        nc.gpsimd.dma_gather(vg, vtab,
                             idx_v[:, blk * BL * 8:(blk + 1) * BL * 8],
                             C * BL, C * BL, H, single_packet=False)
        ctp = ps.tile([C, BL], F32, tag="ps", name="cT_ps")
        nc.tensor.transpose(ctp, cb[:, blk * C:(blk + 1) * C],
                            ident[0:BL, 0:BL])
        cT = sg.tile([C, BL], F32, tag="cT", name="cT")
        nc.vector.tensor_copy(cT, ctp)
        if _LVL >= 3:
            rps_c = rp.tile([H, BL], F32, tag="r", name="rps_c")
            for b in range(BL):
                nc.tensor.matmul(rps_c[:, b:b + 1], vg[:, b, :],
                                 cT[:, b:b + 1],
                                 start=(b == 0), stop=(b == BL - 1))
            nc.vector.tensor_add(racc, racc, rps_c)
        else:
            nc.vector.tensor_add(racc, racc, cT[0:H, :])

    # ---- head: out^T = WO^T (-r)^T + BO ----
    rT = sg.tile([H, BL], F32)
    nc.vector.tensor_scalar(out=rT, in0=racc, scalar1=-1.0, scalar2=None,
                            op0=OP.mult)
    oTp = ps.tile([V, BL], F32, tag="ps")
    nc.tensor.matmul(oTp, WO, rT, start=True, stop=True)   # [v' x b]
    oT = sg.tile([V, BL], F32)
    nc.vector.tensor_scalar(out=oT, in0=oTp, scalar1=BO, scalar2=None,
                            op0=OP.add)
    nc.sync.dma_start(out=d["out_t"], in_=oT)
    ctx.close()


def _build(num_devices=NCORES):
    nc = bacc.Bacc("TRN2", num_devices=num_devices)
    d = {}
    d["seq"] = nc.dram_tensor("seq", [BL, L], I32, kind="ExternalInput").ap()
    shapes = {"embed": [V, H], "w1": [H, 2 * H], "b1": [2 * H], "w2": [2 * H, H],
              "b2": [H], "ln_g": [H], "ln_b": [H], "wk": [H, H], "wv": [H, H],
              "wq": [H, H], "wrp": [H, H], "brp": [H], "wout": [H, V],
              "bout": [V]}
    for n, shp in shapes.items():
        d[n] = nc.dram_tensor(n, shp, F32, kind="ExternalInput").ap()
    d["out_t"] = nc.dram_tensor("out_t", [V, BL], F32,
                                kind="ExternalOutput").ap()
    with tile.TileContext(nc) as tc:
        _body(nc, tc, d)
    nc.compile()
    return nc


def kernel(_trace=False, _tmpdir=None, **inputs):
    if "nc" not in _CACHE:
        _CACHE["nc"] = _build()
    nc = _CACHE["nc"]
    seq = np.ascontiguousarray(np.asarray(inputs["seq"], dtype=np.int32))
    ws = {n: np.ascontiguousarray(np.asarray(inputs[n], dtype=np.float32))
          for n in WNAMES}
    in_maps = []
    for c in range(NCORES):
        m = {"seq": seq[c * BL:(c + 1) * BL]}
        m.update(ws)
        in_maps.append(m)
    res = run_bass_kernel_spmd(nc, in_maps, core_ids=list(range(NCORES)),
                               trace=_trace, tmpdir=_tmpdir)
    _CACHE["last_results"] = res
    out = np.concatenate([r["out_t"].T for r in res.results], axis=0)
    return np.ascontiguousarray(out)
